# revision 31
# baseline (speedup 1.0000x reference)
"""GRU (r_t=1) Trainium2 kernel v7.

vs v6:
- input projections are injected into the persistent PSUM accumulators as
  per-step deltas (identity-stationary fp32 matmuls): psz_acc/psn_acc hold
  tz_t/tn_t directly, so sigmoid/tanh read PSUM with no DVE pre-add. The
  deltas telescope in fp32 (d_t = iz_t - iz_{t-1} from phase-1 psum), so
  no error random-walk across the 512-step accumulation.
- per-step critical path: n-gate matmuls -> tanh -> sub overlap the z-gate
  matmuls; sigmoid -> mul close the loop. Injection MMs are queued first
  (gated only on the previous step's activation reads) so they run in the
  PE-idle tail, as do the interleaved phase-1 matmuls.
"""

import sys

if "/opt/trn_rl_repo" not in sys.path:
    sys.path.insert(0, "/opt/trn_rl_repo")

from contextlib import ExitStack

import ml_dtypes
import numpy as np

import concourse.bacc as bacc
import concourse.mybir as mybir
import concourse.tile as tile
from concourse import bass_utils

NCORES = 8
DIN = 512
DH = 512
CH = DH // 128
AF = mybir.ActivationFunctionType
BF16 = mybir.dt.bfloat16
F32 = mybir.dt.float32
NPBF16 = np.dtype(ml_dtypes.bfloat16)


def build_nc(T: int, BC: int):
    R = T * BC
    D = CH * BC  # flat per-step element count per partition (64)
    PCOL = 512
    assert R % PCOL == 0
    NT1 = R // PCOL
    CSTEPS = PCOL // BC  # scan steps covered by one phase-1 chunk (32)
    BLK = CSTEPS
    assert T % BLK == 0
    NBLK = T // BLK
    PRO = min(2, NT1)  # prologue chunks

    nc = bacc.Bacc("TRN2", target_bir_lowering=False, debug=False)

    seqT = nc.dram_tensor("seqT", [DIN, R], BF16, kind="ExternalInput").ap()
    wizT = nc.dram_tensor("wizT", [DIN, DH], BF16, kind="ExternalInput").ap()
    winT = nc.dram_tensor("winT", [DIN, DH], BF16, kind="ExternalInput").ap()
    whzT = nc.dram_tensor("whzT", [DH, DH], BF16, kind="ExternalInput").ap()
    whnT = nc.dram_tensor("whnT", [DH, DH], BF16, kind="ExternalInput").ap()
    biasz = nc.dram_tensor("biasz", [128, CH], F32, kind="ExternalInput").ap()
    biasn = nc.dram_tensor("biasn", [128, CH], F32, kind="ExternalInput").ap()
    ident = nc.dram_tensor("ident", [128, 128], BF16, kind="ExternalInput").ap()
    # time-major flat output: HT[p, t, m*BC+b] ; h[m*128+p] at (t, b)
    HT = nc.dram_tensor("HT", [128, T, D], F32, kind="ExternalOutput").ap()

    with tile.TileContext(nc) as tc, ExitStack() as ctx:
        const = ctx.enter_context(tc.tile_pool(name="const", bufs=1))

        wiz_sb = const.tile([128, CH, DH], BF16)
        win_sb = const.tile([128, CH, DH], BF16)
        whz_sb = const.tile([128, CH, DH], BF16)
        whn_sb = const.tile([128, CH, DH], BF16)
        for sb, dr in ((wiz_sb, wizT), (win_sb, winT), (whz_sb, whzT), (whn_sb, whnT)):
            nc.gpsimd.dma_start(sb[:], dr.rearrange("(c p) h -> p c h", p=128))
        bz_sb = const.tile([128, CH], F32)
        bn_sb = const.tile([128, CH], F32)
        id_sb = const.tile([128, 128], BF16)
        nc.gpsimd.dma_start(bz_sb[:], biasz[:])
        nc.gpsimd.dma_start(bn_sb[:], biasn[:])
        nc.gpsimd.dma_start(id_sb[:], ident[:])
        # last-step input projections of the previous chunk, per group (fp32)
        izlast = const.tile([128, 2 * CH, BC], F32)

        junk_ps = ctx.enter_context(tc.tile_pool(name="junkps", bufs=1, space="PSUM"))
        junk = junk_ps.tile([128, 8], F32)
        scratch = const.tile([128, 8], F32)

        def pe_touch(ap_k1):
            nc.tensor.matmul(junk[0:1, 0:1], ap_k1, ap_k1, start=True, stop=True)

        for sb in (wiz_sb, win_sb, whz_sb, whn_sb):
            pe_touch(sb[:, 0, 0:1])
        nc.vector.tensor_copy(scratch[0:1, 0:1], bz_sb[0:1, 0:1])
        nc.vector.tensor_copy(scratch[0:1, 1:2], bn_sb[0:1, 1:2])
        nc.vector.tensor_copy(scratch[0:1, 2:3], id_sb[0:1, 0:1])

        seq_pool = ctx.enter_context(tc.tile_pool(name="seqp", bufs=3))
        psum1 = ctx.enter_context(tc.tile_pool(name="psum1", bufs=2, space="PSUM"))
        # per-chunk input-projection deltas, time-major [128, step, m*BC+b].
        # fp32 master plus bf16 hi/lo split (the injection matmuls are bf16:
        # fp32 matmuls measure ~450ns each, bf16 ~27ns; hi+lo keeps the
        # telescoped sum exact to ~2^-18).
        dz_pool = ctx.enter_context(tc.tile_pool(name="dzp", bufs=3))
        dn_pool = ctx.enter_context(tc.tile_pool(name="dnp", bufs=3))

        def chunk_dma(c):
            # no pe_touch here: a junk matmul gated on this DMA would
            # head-of-line-block the PE queue for the DMA's full latency.
            csl = slice(c * PCOL, (c + 1) * PCOL)
            sq = seq_pool.tile([128, CH, PCOL], BF16, tag="sq")
            nc.gpsimd.dma_start(sq[:], seqT[:, csl].rearrange("(c p) r -> p c r", p=128))
            return sq

        def new_delta_tiles():
            dz = dz_pool.tile([128, CSTEPS, D], F32, tag="dz")
            dn = dn_pool.tile([128, CSTEPS, D], F32, tag="dn")
            dzh = dz_pool.tile([128, CSTEPS, D], BF16, tag="dzh")
            dzl = dz_pool.tile([128, CSTEPS, D], BF16, tag="dzl")
            dnh = dn_pool.tile([128, CSTEPS, D], BF16, tag="dnh")
            dnl = dn_pool.tile([128, CSTEPS, D], BF16, tag="dnl")
            return (dz, dn, dzh, dzl, dnh, dnl)

        def p1_group_mm(sq, ps, g, k):
            m = g % CH
            w_sb = wiz_sb if g < CH else win_sb
            nc.tensor.matmul(
                ps[:],
                w_sb[:, k, m * 128 : (m + 1) * 128],
                sq[:, k, :],
                start=(k == 0),
                stop=(k == CH - 1),
            )

        izsb_pool = ctx.enter_context(tc.tile_pool(name="izsb", bufs=2))

        def p1_group_delta_thunks(ps, c, g, dzdn):
            """Per-chunk delta pipeline for group g, as 6 single-op thunks.

            ps columns are (step, batch) flat: col = s*BC + b, s in [0, CSTEPS).
            d[s] = proj[s] - proj[s-1]; cross-chunk boundary via izlast[g];
            chunk 0 uses the (combined) bias instead: d[0] = proj[0] + b.
            DVE reads at most one PSUM input per op, so ps is staged to SBUF.
            The fp32 delta is then split into bf16 hi + lo for injection.
            """
            m = g % CH
            dst = dzdn[0] if g < CH else dzdn[1]
            hi = dzdn[2] if g < CH else dzdn[4]
            lo = dzdn[3] if g < CH else dzdn[5]
            msl = slice(m * BC, (m + 1) * BC)
            izsb = izsb_pool.tile([128, PCOL], F32, tag="izsb")

            def t_copy():
                nc.vector.tensor_copy(izsb[:], ps[:])

            def t_shift_sub():
                nc.vector.tensor_sub(
                    dst[:, 1:CSTEPS, msl],
                    izsb[:, BC:PCOL],
                    izsb[:, 0 : PCOL - BC],
                )

            def t_boundary():
                if c == 0:
                    b_sb = bz_sb if g < CH else bn_sb
                    nc.vector.tensor_scalar_add(
                        dst[:, 0, msl], izsb[:, 0:BC], b_sb[:, m : m + 1]
                    )
                else:
                    nc.vector.tensor_sub(
                        dst[:, 0, msl], izsb[:, 0:BC], izlast[:, g, :]
                    )

            def t_izlast():
                nc.vector.tensor_copy(izlast[:, g, :], izsb[:, PCOL - BC : PCOL])

            def t_hi():
                nc.vector.tensor_copy(hi[:, :, msl], dst[:, :, msl])

            def t_lo():
                nc.vector.tensor_sub(lo[:, :, msl], dst[:, :, msl], hi[:, :, msl])

            return [t_copy, t_shift_sub, t_boundary, t_izlast, t_hi, t_lo]

        def p1_group_delta(ps, c, g, dzdn):
            for th in p1_group_delta_thunks(ps, c, g, dzdn):
                th()

        delta_tiles = [new_delta_tiles()]
        for c in range(PRO):
            if c > 0:
                delta_tiles.append(new_delta_tiles())
            sq = chunk_dma(c)
            for g in range(2 * CH):
                ps = psum1.tile([128, PCOL], F32, tag="ps1")
                for k in range(CH):
                    p1_group_mm(sq, ps, g, k)
                p1_group_delta(ps, c, g, delta_tiles[c])
        # prefetch one block ahead: chunk pipe_c's DMA is issued a full block
        # before its phase-1 matmuls run, so they never wait on it.
        sq_next = chunk_dma(PRO) if PRO < NT1 else None

        with (
            tc.tile_pool(name="ht2", bufs=2) as ht_pool,
            tc.tile_pool(name="st2", bufs=2) as state_pool,
            tc.tile_pool(name="ew2", bufs=2) as ew_pool,
            tc.tile_pool(name="accp", bufs=1, space="PSUM") as accp,
        ):
            # zero bf16 tile: moving operand for the step-0 matmuls
            z0 = state_pool.tile([128, D], BF16, tag="z0")
            nc.vector.memset(z0[:], 0.0)
            h0 = state_pool.tile([128, D], F32, tag="h0")
            nc.vector.memset(h0[:], 0.0)
            h = h0
            d2_prev = z0

            # persistent cross-step accumulators: ps_t = inproj_t + W @ h_{t-1}
            psn_acc = accp.tile([128, D], F32, tag="psn_acc")
            psz_acc = accp.tile([128, D], F32, tag="psz_acc")

            p1_sq = None
            p1_ps = None
            p1_pending = []  # deferred (ps, c, g) delta emissions

            for blk in range(NBLK):
                htb = ht_pool.tile([128, BLK, D], F32, tag="htb")
                _, _, dzh_blk, dzl_blk, dnh_blk, dnl_blk = delta_tiles[blk]
                pipe_c = blk + PRO
                if pipe_c < NT1:
                    p1_sq = sq_next
                    if pipe_c + 1 < NT1:
                        sq_next = chunk_dma(pipe_c + 1)
                    delta_tiles.append(new_delta_tiles())

                for tl in range(BLK):
                    t = blk * BLK + tl
                    # inject input-projection deltas (bf16 hi/lo identity
                    # matmuls); n first: gated only on the earlier tanh read.
                    for acc, hi_t, lo_t in (
                        (psn_acc, dnh_blk, dnl_blk),
                        (psz_acc, dzh_blk, dzl_blk),
                    ):
                        nc.tensor.matmul(
                            acc[:],
                            id_sb[:],
                            hi_t[:, tl, :],
                            start=(t == 0),
                            stop=False,
                            skip_group_check=True,
                        )
                        nc.tensor.matmul(
                            acc[:],
                            id_sb[:],
                            lo_t[:, tl, :],
                            start=False,
                            stop=False,
                            skip_group_check=True,
                        )
                    # recurrent matmuls: n gate first (tanh unblocks at half
                    # burst), z gate second.
                    for w_sb, ps in ((whn_sb, psn_acc), (whz_sb, psz_acc)):
                        for m in range(CH):
                            for k in range(CH):
                                nc.tensor.matmul(
                                    ps[:, m * BC : (m + 1) * BC],
                                    w_sb[:, k, m * 128 : (m + 1) * 128],
                                    d2_prev[:, k * BC : (k + 1) * BC],
                                    start=False,
                                    stop=(t == T - 1),
                                    skip_group_check=True,
                                )
                    # phase-1 interleave: one matmul per step fills the tail
                    if pipe_c < NT1 and tl < 8 * CH:
                        g, k = tl // CH, tl % CH
                        if k == 0:
                            p1_ps = psum1.tile([128, PCOL], F32, tag="ps1")
                        p1_group_mm(p1_sq, p1_ps, g, k)
                        if k == CH - 1:
                            # eligible from the NEXT step on: the group's last
                            # phase-1 matmul only runs in this step's tail, and
                            # t_copy waiting on it would head-of-line-block the
                            # DVE queue.
                            p1_pending.extend(
                                (t + 1, th)
                                for th in p1_group_delta_thunks(
                                    p1_ps, pipe_c, g, delta_tiles[pipe_c]
                                )
                            )

                    # n path: tanh reads the accumulator straight from PSUM
                    nt = ew_pool.tile([128, D], BF16, tag="nt")
                    nc.scalar.activation(nt[:], psn_acc[:], AF.Tanh)
                    d = ew_pool.tile([128, D], BF16, tag="d")
                    nc.vector.tensor_sub(d[:], nt[:], h[:])
                    # z path: h_new = h + sigmoid(-tz)*(n - h)
                    zc = ew_pool.tile([128, D], BF16, tag="zc")
                    nc.scalar.activation(zc[:], psz_acc[:], AF.Sigmoid, scale=-1.0)
                    d2 = ew_pool.tile([128, D], BF16, tag="d2")
                    nc.vector.tensor_mul(d2[:], zc[:], d[:])
                    hn = htb[:, tl, :]
                    nc.vector.tensor_add(hn, h[:], d2[:])
                    # HAM-warming filler: keeps the PE active through the
                    # elementwise tail so the clock gate stays at 8/8. Emitted
                    # after the activations so their semaphore thresholds
                    # exclude it.
                    for _ in range(6):
                        pe_touch(whz_sb[:, 0, 0:1])
                    emitted = 0
                    while p1_pending and p1_pending[0][0] <= t and emitted < 2:
                        p1_pending.pop(0)[1]()
                        emitted += 1
                    h = hn
                    d2_prev = d2

                nc.gpsimd.dma_start(HT[:, blk * BLK : (blk + 1) * BLK, :], htb[:])

    nc.compile()
    return nc


_CACHE: dict = {}


def _get_nc(T, BC):
    key = (T, BC)
    if key not in _CACHE:
        _CACHE[key] = build_nc(T, BC)
    return _CACHE[key]


def _in_maps(inputs, T, BC):
    f32 = np.float32
    wizT = np.ascontiguousarray(np.asarray(inputs["W_iz"], f32).T.astype(NPBF16))
    winT = np.ascontiguousarray(np.asarray(inputs["W_in"], f32).T.astype(NPBF16))
    whzT = np.ascontiguousarray(np.asarray(inputs["W_hz"], f32).T.astype(NPBF16))
    whnT = np.ascontiguousarray(np.asarray(inputs["W_hn"], f32).T.astype(NPBF16))
    biasz = np.ascontiguousarray(
        (np.asarray(inputs["b_iz"], f32) + np.asarray(inputs["b_hz"], f32)).reshape(CH, 128).T
    )
    biasn = np.ascontiguousarray(
        (np.asarray(inputs["b_in"], f32) + np.asarray(inputs["b_hn"], f32)).reshape(CH, 128).T
    )
    ident = np.eye(128, dtype=f32).astype(NPBF16)
    seq = np.asarray(inputs["seq"], f32)
    in_maps = []
    for c in range(NCORES):
        shard = seq[:, c * BC : (c + 1) * BC, :].reshape(T * BC, DIN)
        seqT = np.ascontiguousarray(shard.T.astype(NPBF16))
        in_maps.append(
            {
                "seqT": seqT,
                "wizT": wizT,
                "winT": winT,
                "whzT": whzT,
                "whnT": whnT,
                "biasz": biasz,
                "biasn": biasn,
                "ident": ident,
            }
        )
    return in_maps


def _assemble(results, T, BC):
    out = np.empty((T, NCORES * BC, DH), np.float32)
    for c in range(NCORES):
        HT = np.asarray(results[c]["HT"], dtype=np.float32)  # [128, T, CH*BC]
        # HT[p, t, m*BC+b] -> out[t, b, m*128+p]
        Hc = HT.reshape(128, T, CH, BC).transpose(1, 3, 2, 0).reshape(T, BC, DH)
        out[:, c * BC : (c + 1) * BC, :] = Hc
    return out[None]


def kernel(seq, W_iz, b_iz, W_in, b_in, W_hz, b_hz, W_hn, b_hn):
    seq = np.asarray(seq)
    T, B, _ = seq.shape
    BC = B // NCORES
    nc = _get_nc(T, BC)
    in_maps = _in_maps(
        dict(seq=seq, W_iz=W_iz, b_iz=b_iz, W_in=W_in, b_in=b_in,
             W_hz=W_hz, b_hz=b_hz, W_hn=W_hn, b_hn=b_hn),
        T, BC,
    )
    res = bass_utils.run_bass_kernel_spmd(nc, in_maps, list(range(NCORES)))
    return _assemble(res.results, T, BC)


def traced_run(inputs):
    seq = np.asarray(inputs["seq"])
    T, B, _ = seq.shape
    BC = B // NCORES
    nc = _get_nc(T, BC)
    in_maps = _in_maps(inputs, T, BC)
    return bass_utils.run_bass_kernel_spmd(
        nc, in_maps, list(range(NCORES)), trace=True
    )


# revision 32
# speedup vs baseline: 1.0949x; 1.0949x over previous
"""GRU (r_t=1) Trainium2 kernel v7.

vs v6:
- input projections are injected into the persistent PSUM accumulators as
  per-step deltas (identity-stationary fp32 matmuls): psz_acc/psn_acc hold
  tz_t/tn_t directly, so sigmoid/tanh read PSUM with no DVE pre-add. The
  deltas telescope in fp32 (d_t = iz_t - iz_{t-1} from phase-1 psum), so
  no error random-walk across the 512-step accumulation.
- per-step critical path: n-gate matmuls -> tanh -> sub overlap the z-gate
  matmuls; sigmoid -> mul close the loop. Injection MMs are queued first
  (gated only on the previous step's activation reads) so they run in the
  PE-idle tail, as do the interleaved phase-1 matmuls.
"""

import sys

if "/opt/trn_rl_repo" not in sys.path:
    sys.path.insert(0, "/opt/trn_rl_repo")

from contextlib import ExitStack

import ml_dtypes
import numpy as np

import concourse.bacc as bacc
import concourse.mybir as mybir
import concourse.tile as tile
from concourse import bass_utils

NCORES = 8
DIN = 512
DH = 512
CH = DH // 128
AF = mybir.ActivationFunctionType
BF16 = mybir.dt.bfloat16
F32 = mybir.dt.float32
NPBF16 = np.dtype(ml_dtypes.bfloat16)


def build_nc(T: int, BC: int):
    R = T * BC
    D = CH * BC  # flat per-step element count per partition (64)
    PCOL = 512
    assert R % PCOL == 0
    NT1 = R // PCOL
    CSTEPS = PCOL // BC  # scan steps covered by one phase-1 chunk (32)
    BLK = CSTEPS
    assert T % BLK == 0
    NBLK = T // BLK
    PRO = min(2, NT1)  # prologue chunks

    nc = bacc.Bacc("TRN2", target_bir_lowering=False, debug=False)

    seqT = nc.dram_tensor("seqT", [DIN, R], BF16, kind="ExternalInput").ap()
    wizT = nc.dram_tensor("wizT", [DIN, DH], BF16, kind="ExternalInput").ap()
    winT = nc.dram_tensor("winT", [DIN, DH], BF16, kind="ExternalInput").ap()
    whzT = nc.dram_tensor("whzT", [DH, DH], BF16, kind="ExternalInput").ap()
    whnT = nc.dram_tensor("whnT", [DH, DH], BF16, kind="ExternalInput").ap()
    biasz = nc.dram_tensor("biasz", [128, CH], F32, kind="ExternalInput").ap()
    biasn = nc.dram_tensor("biasn", [128, CH], F32, kind="ExternalInput").ap()
    ident = nc.dram_tensor("ident", [128, 128], BF16, kind="ExternalInput").ap()
    # time-major flat output: HT[p, t, m*BC+b] ; h[m*128+p] at (t, b)
    HT = nc.dram_tensor("HT", [128, T, D], F32, kind="ExternalOutput").ap()

    with tile.TileContext(nc) as tc, ExitStack() as ctx:
        const = ctx.enter_context(tc.tile_pool(name="const", bufs=1))

        wiz_sb = const.tile([128, CH, DH], BF16)
        win_sb = const.tile([128, CH, DH], BF16)
        whz_sb = const.tile([128, CH, DH], BF16)
        whn_sb = const.tile([128, CH, DH], BF16)
        for sb, dr in ((wiz_sb, wizT), (win_sb, winT), (whz_sb, whzT), (whn_sb, whnT)):
            nc.gpsimd.dma_start(sb[:], dr.rearrange("(c p) h -> p c h", p=128))
        bz_sb = const.tile([128, CH], F32)
        bn_sb = const.tile([128, CH], F32)
        id_sb = const.tile([128, 128], BF16)
        nc.gpsimd.dma_start(bz_sb[:], biasz[:])
        nc.gpsimd.dma_start(bn_sb[:], biasn[:])
        nc.gpsimd.dma_start(id_sb[:], ident[:])
        # last-step input projections of the previous chunk, per group (fp32)
        izlast = const.tile([128, 2 * CH, BC], F32)

        junk_ps = ctx.enter_context(tc.tile_pool(name="junkps", bufs=1, space="PSUM"))
        junk = junk_ps.tile([128, 8], F32)
        scratch = const.tile([128, 8], F32)

        def pe_touch(ap_k1):
            nc.tensor.matmul(junk[0:1, 0:1], ap_k1, ap_k1, start=True, stop=True)

        for sb in (wiz_sb, win_sb, whz_sb, whn_sb):
            pe_touch(sb[:, 0, 0:1])
        nc.vector.tensor_copy(scratch[0:1, 0:1], bz_sb[0:1, 0:1])
        nc.vector.tensor_copy(scratch[0:1, 1:2], bn_sb[0:1, 1:2])
        nc.vector.tensor_copy(scratch[0:1, 2:3], id_sb[0:1, 0:1])

        seq_pool = ctx.enter_context(tc.tile_pool(name="seqp", bufs=3))
        psum1 = ctx.enter_context(tc.tile_pool(name="psum1", bufs=2, space="PSUM"))
        # per-chunk input-projection deltas, time-major [128, step, m*BC+b].
        # fp32 master plus bf16 hi/lo split (the injection matmuls are bf16:
        # fp32 matmuls measure ~450ns each, bf16 ~27ns; hi+lo keeps the
        # telescoped sum exact to ~2^-18).
        dz_pool = ctx.enter_context(tc.tile_pool(name="dzp", bufs=3))
        dn_pool = ctx.enter_context(tc.tile_pool(name="dnp", bufs=3))

        def chunk_dma(c):
            # no pe_touch here: a junk matmul gated on this DMA would
            # head-of-line-block the PE queue for the DMA's full latency.
            csl = slice(c * PCOL, (c + 1) * PCOL)
            sq = seq_pool.tile([128, CH, PCOL], BF16, tag="sq")
            nc.gpsimd.dma_start(sq[:], seqT[:, csl].rearrange("(c p) r -> p c r", p=128))
            return sq

        def new_delta_tiles():
            dz = dz_pool.tile([128, CSTEPS, D], F32, tag="dz")
            dn = dn_pool.tile([128, CSTEPS, D], F32, tag="dn")
            dzh = dz_pool.tile([128, CSTEPS, D], BF16, tag="dzh")
            dzl = dz_pool.tile([128, CSTEPS, D], BF16, tag="dzl")
            dnh = dn_pool.tile([128, CSTEPS, D], BF16, tag="dnh")
            dnl = dn_pool.tile([128, CSTEPS, D], BF16, tag="dnl")
            return (dz, dn, dzh, dzl, dnh, dnl)

        def p1_group_mm(sq, ps, g, k):
            m = g % CH
            w_sb = wiz_sb if g < CH else win_sb
            nc.tensor.matmul(
                ps[:],
                w_sb[:, k, m * 128 : (m + 1) * 128],
                sq[:, k, :],
                start=(k == 0),
                stop=(k == CH - 1),
            )

        izsb_pool = ctx.enter_context(tc.tile_pool(name="izsb", bufs=2))

        def p1_group_delta_thunks(ps, c, g, dzdn):
            """Per-chunk delta pipeline for group g, as 6 single-op thunks.

            ps columns are (step, batch) flat: col = s*BC + b, s in [0, CSTEPS).
            d[s] = proj[s] - proj[s-1]; cross-chunk boundary via izlast[g];
            chunk 0 uses the (combined) bias instead: d[0] = proj[0] + b.
            DVE reads at most one PSUM input per op, so ps is staged to SBUF.
            The fp32 delta is then split into bf16 hi + lo for injection.
            """
            m = g % CH
            dst = dzdn[0] if g < CH else dzdn[1]
            hi = dzdn[2] if g < CH else dzdn[4]
            lo = dzdn[3] if g < CH else dzdn[5]
            msl = slice(m * BC, (m + 1) * BC)
            izsb = izsb_pool.tile([128, PCOL], F32, tag="izsb")

            def t_copy():
                nc.vector.tensor_copy(izsb[:], ps[:])

            def t_shift_sub():
                nc.vector.tensor_sub(
                    dst[:, 1:CSTEPS, msl],
                    izsb[:, BC:PCOL],
                    izsb[:, 0 : PCOL - BC],
                )

            def t_boundary():
                if c == 0:
                    b_sb = bz_sb if g < CH else bn_sb
                    nc.vector.tensor_scalar_add(
                        dst[:, 0, msl], izsb[:, 0:BC], b_sb[:, m : m + 1]
                    )
                else:
                    nc.vector.tensor_sub(
                        dst[:, 0, msl], izsb[:, 0:BC], izlast[:, g, :]
                    )

            def t_izlast():
                nc.vector.tensor_copy(izlast[:, g, :], izsb[:, PCOL - BC : PCOL])

            def t_hi():
                nc.vector.tensor_copy(hi[:, :, msl], dst[:, :, msl])

            def t_lo():
                nc.vector.tensor_sub(lo[:, :, msl], dst[:, :, msl], hi[:, :, msl])

            return [t_copy, t_shift_sub, t_boundary, t_izlast, t_hi, t_lo]

        def p1_group_delta(ps, c, g, dzdn):
            for th in p1_group_delta_thunks(ps, c, g, dzdn):
                th()

        delta_tiles = [new_delta_tiles()]
        for c in range(PRO):
            if c > 0:
                delta_tiles.append(new_delta_tiles())
            sq = chunk_dma(c)
            for g in range(2 * CH):
                ps = psum1.tile([128, PCOL], F32, tag="ps1")
                for k in range(CH):
                    p1_group_mm(sq, ps, g, k)
                p1_group_delta(ps, c, g, delta_tiles[c])
        # prefetch one block ahead: chunk pipe_c's DMA is issued a full block
        # before its phase-1 matmuls run, so they never wait on it.
        sq_next = chunk_dma(PRO) if PRO < NT1 else None

        with (
            tc.tile_pool(name="ht2", bufs=2) as ht_pool,
            tc.tile_pool(name="st2", bufs=2) as state_pool,
            tc.tile_pool(name="ew2", bufs=2) as ew_pool,
            tc.tile_pool(name="accp", bufs=1, space="PSUM") as accp,
        ):
            # zero bf16 tile: moving operand for the step-0 matmuls
            z0 = state_pool.tile([128, D], BF16, tag="z0")
            nc.vector.memset(z0[:], 0.0)
            h0 = state_pool.tile([128, D], F32, tag="h0")
            nc.vector.memset(h0[:], 0.0)
            h = h0
            d2_prev = z0

            # persistent cross-step accumulators: ps_t = inproj_t + W @ h_{t-1}
            psn_acc = accp.tile([128, D], F32, tag="psn_acc")
            psz_acc = accp.tile([128, D], F32, tag="psz_acc")

            p1_sq = None
            p1_ps = None
            p1_pending = []  # deferred (ps, c, g) delta emissions

            for blk in range(NBLK):
                htb = ht_pool.tile([128, BLK, D], F32, tag="htb")
                _, _, dzh_blk, dzl_blk, dnh_blk, dnl_blk = delta_tiles[blk]
                pipe_c = blk + PRO
                if pipe_c < NT1:
                    p1_sq = sq_next
                    if pipe_c + 1 < NT1:
                        sq_next = chunk_dma(pipe_c + 1)
                    delta_tiles.append(new_delta_tiles())

                for tl in range(BLK):
                    t = blk * BLK + tl
                    # inject input-projection deltas (bf16 hi/lo identity
                    # matmuls); n first: gated only on the earlier tanh read.
                    for acc, hi_t, lo_t in (
                        (psn_acc, dnh_blk, dnl_blk),
                        (psz_acc, dzh_blk, dzl_blk),
                    ):
                        nc.tensor.matmul(
                            acc[:],
                            id_sb[:],
                            hi_t[:, tl, :],
                            start=(t == 0),
                            stop=False,
                            skip_group_check=True,
                        )
                        nc.tensor.matmul(
                            acc[:],
                            id_sb[:],
                            lo_t[:, tl, :],
                            start=False,
                            stop=False,
                            skip_group_check=True,
                        )
                    # recurrent matmuls: n gate first (tanh unblocks at half
                    # burst), z gate second.
                    for w_sb, ps in ((whn_sb, psn_acc), (whz_sb, psz_acc)):
                        for m in range(CH):
                            for k in range(CH):
                                nc.tensor.matmul(
                                    ps[:, m * BC : (m + 1) * BC],
                                    w_sb[:, k, m * 128 : (m + 1) * 128],
                                    d2_prev[:, k * BC : (k + 1) * BC],
                                    start=False,
                                    stop=(t == T - 1),
                                    skip_group_check=True,
                                )
                    # phase-1 interleave: one matmul per step fills the tail
                    if pipe_c < NT1 and tl < 8 * CH:
                        g, k = tl // CH, tl % CH
                        if k == 0:
                            p1_ps = psum1.tile([128, PCOL], F32, tag="ps1")
                        p1_group_mm(p1_sq, p1_ps, g, k)
                        if k == CH - 1:
                            # eligible from the NEXT step on: the group's last
                            # phase-1 matmul only runs in this step's tail, and
                            # t_copy waiting on it would head-of-line-block the
                            # DVE queue.
                            p1_pending.extend(
                                (t + 1, th)
                                for th in p1_group_delta_thunks(
                                    p1_ps, pipe_c, g, delta_tiles[pipe_c]
                                )
                            )

                    # n path: tanh reads the accumulator straight from PSUM
                    nt = ew_pool.tile([128, D], BF16, tag="nt")
                    nc.scalar.activation(nt[:], psn_acc[:], AF.Tanh)
                    d = ew_pool.tile([128, D], BF16, tag="d")
                    nc.vector.tensor_sub(d[:], nt[:], h[:])
                    # z path: h_new = h + sigmoid(-tz)*(n - h)
                    zc = ew_pool.tile([128, D], BF16, tag="zc")
                    nc.scalar.activation(zc[:], psz_acc[:], AF.Sigmoid, scale=-1.0)
                    d2 = ew_pool.tile([128, D], BF16, tag="d2")
                    nc.vector.tensor_mul(d2[:], zc[:], d[:])
                    hn = htb[:, tl, :]
                    nc.vector.tensor_add(hn, h[:], d2[:])
                    emitted = 0
                    while p1_pending and p1_pending[0][0] <= t and emitted < 2:
                        p1_pending.pop(0)[1]()
                        emitted += 1
                    h = hn
                    d2_prev = d2

                nc.gpsimd.dma_start(HT[:, blk * BLK : (blk + 1) * BLK, :], htb[:])

    nc.compile()
    return nc


_CACHE: dict = {}


def _get_nc(T, BC):
    key = (T, BC)
    if key not in _CACHE:
        _CACHE[key] = build_nc(T, BC)
    return _CACHE[key]


def _in_maps(inputs, T, BC):
    f32 = np.float32
    wizT = np.ascontiguousarray(np.asarray(inputs["W_iz"], f32).T.astype(NPBF16))
    winT = np.ascontiguousarray(np.asarray(inputs["W_in"], f32).T.astype(NPBF16))
    whzT = np.ascontiguousarray(np.asarray(inputs["W_hz"], f32).T.astype(NPBF16))
    whnT = np.ascontiguousarray(np.asarray(inputs["W_hn"], f32).T.astype(NPBF16))
    biasz = np.ascontiguousarray(
        (np.asarray(inputs["b_iz"], f32) + np.asarray(inputs["b_hz"], f32)).reshape(CH, 128).T
    )
    biasn = np.ascontiguousarray(
        (np.asarray(inputs["b_in"], f32) + np.asarray(inputs["b_hn"], f32)).reshape(CH, 128).T
    )
    ident = np.eye(128, dtype=f32).astype(NPBF16)
    seq = np.asarray(inputs["seq"], f32)
    in_maps = []
    for c in range(NCORES):
        shard = seq[:, c * BC : (c + 1) * BC, :].reshape(T * BC, DIN)
        seqT = np.ascontiguousarray(shard.T.astype(NPBF16))
        in_maps.append(
            {
                "seqT": seqT,
                "wizT": wizT,
                "winT": winT,
                "whzT": whzT,
                "whnT": whnT,
                "biasz": biasz,
                "biasn": biasn,
                "ident": ident,
            }
        )
    return in_maps


def _assemble(results, T, BC):
    out = np.empty((T, NCORES * BC, DH), np.float32)
    for c in range(NCORES):
        HT = np.asarray(results[c]["HT"], dtype=np.float32)  # [128, T, CH*BC]
        # HT[p, t, m*BC+b] -> out[t, b, m*128+p]
        Hc = HT.reshape(128, T, CH, BC).transpose(1, 3, 2, 0).reshape(T, BC, DH)
        out[:, c * BC : (c + 1) * BC, :] = Hc
    return out[None]


def kernel(seq, W_iz, b_iz, W_in, b_in, W_hz, b_hz, W_hn, b_hn):
    seq = np.asarray(seq)
    T, B, _ = seq.shape
    BC = B // NCORES
    nc = _get_nc(T, BC)
    in_maps = _in_maps(
        dict(seq=seq, W_iz=W_iz, b_iz=b_iz, W_in=W_in, b_in=b_in,
             W_hz=W_hz, b_hz=b_hz, W_hn=W_hn, b_hn=b_hn),
        T, BC,
    )
    res = bass_utils.run_bass_kernel_spmd(nc, in_maps, list(range(NCORES)))
    return _assemble(res.results, T, BC)


def traced_run(inputs):
    seq = np.asarray(inputs["seq"])
    T, B, _ = seq.shape
    BC = B // NCORES
    nc = _get_nc(T, BC)
    in_maps = _in_maps(inputs, T, BC)
    return bass_utils.run_bass_kernel_spmd(
        nc, in_maps, list(range(NCORES)), trace=True
    )


# revision 35
# speedup vs baseline: 1.1212x; 1.0240x over previous
"""GRU (r_t=1) Trainium2 kernel v7.

vs v6:
- input projections are injected into the persistent PSUM accumulators as
  per-step deltas (identity-stationary fp32 matmuls): psz_acc/psn_acc hold
  tz_t/tn_t directly, so sigmoid/tanh read PSUM with no DVE pre-add. The
  deltas telescope in fp32 (d_t = iz_t - iz_{t-1} from phase-1 psum), so
  no error random-walk across the 512-step accumulation.
- per-step critical path: n-gate matmuls -> tanh -> sub overlap the z-gate
  matmuls; sigmoid -> mul close the loop. Injection MMs are queued first
  (gated only on the previous step's activation reads) so they run in the
  PE-idle tail, as do the interleaved phase-1 matmuls.
"""

import sys

if "/opt/trn_rl_repo" not in sys.path:
    sys.path.insert(0, "/opt/trn_rl_repo")

from contextlib import ExitStack

import ml_dtypes
import numpy as np

import concourse.bacc as bacc
import concourse.mybir as mybir
import concourse.tile as tile
from concourse import bass_utils

NCORES = 8
DIN = 512
DH = 512
CH = DH // 128
AF = mybir.ActivationFunctionType
BF16 = mybir.dt.bfloat16
F32 = mybir.dt.float32
NPBF16 = np.dtype(ml_dtypes.bfloat16)


def build_nc(T: int, BC: int):
    R = T * BC
    D = CH * BC  # flat per-step element count per partition (64)
    PCOL = 512
    assert R % PCOL == 0
    NT1 = R // PCOL
    CSTEPS = PCOL // BC  # scan steps covered by one phase-1 chunk (32)
    BLK = CSTEPS
    assert T % BLK == 0
    NBLK = T // BLK
    PRO = min(2, NT1)  # prologue chunks

    nc = bacc.Bacc("TRN2", target_bir_lowering=False, debug=False)

    seqT = nc.dram_tensor("seqT", [DIN, R], BF16, kind="ExternalInput").ap()
    wizT = nc.dram_tensor("wizT", [DIN, DH], BF16, kind="ExternalInput").ap()
    winT = nc.dram_tensor("winT", [DIN, DH], BF16, kind="ExternalInput").ap()
    whzT = nc.dram_tensor("whzT", [DH, DH], BF16, kind="ExternalInput").ap()
    whnT = nc.dram_tensor("whnT", [DH, DH], BF16, kind="ExternalInput").ap()
    biasz = nc.dram_tensor("biasz", [128, CH], F32, kind="ExternalInput").ap()
    biasn = nc.dram_tensor("biasn", [128, CH], F32, kind="ExternalInput").ap()
    ident = nc.dram_tensor("ident", [128, 128], BF16, kind="ExternalInput").ap()
    # time-major flat output: HT[p, t, m*BC+b] ; h[m*128+p] at (t, b)
    HT = nc.dram_tensor("HT", [128, T, D], F32, kind="ExternalOutput").ap()

    with tile.TileContext(nc) as tc, ExitStack() as ctx:
        const = ctx.enter_context(tc.tile_pool(name="const", bufs=1))

        wiz_sb = const.tile([128, CH, DH], BF16)
        win_sb = const.tile([128, CH, DH], BF16)
        whz_sb = const.tile([128, CH, DH], BF16)
        whn_sb = const.tile([128, CH, DH], BF16)
        for sb, dr in ((wiz_sb, wizT), (win_sb, winT), (whz_sb, whzT), (whn_sb, whnT)):
            nc.gpsimd.dma_start(sb[:], dr.rearrange("(c p) h -> p c h", p=128))
        bz_sb = const.tile([128, CH], F32)
        bn_sb = const.tile([128, CH], F32)
        id_sb = const.tile([128, 128], BF16)
        nc.gpsimd.dma_start(bz_sb[:], biasz[:])
        nc.gpsimd.dma_start(bn_sb[:], biasn[:])
        nc.gpsimd.dma_start(id_sb[:], ident[:])
        # last-step input projections of the previous chunk, per group (fp32)
        izlast = const.tile([128, 2 * CH, BC], F32)

        junk_ps = ctx.enter_context(tc.tile_pool(name="junkps", bufs=1, space="PSUM"))
        junk = junk_ps.tile([128, 8], F32)
        scratch = const.tile([128, 8], F32)

        def pe_touch(ap_k1):
            nc.tensor.matmul(junk[0:1, 0:1], ap_k1, ap_k1, start=True, stop=True)

        for sb in (wiz_sb, win_sb, whz_sb, whn_sb):
            pe_touch(sb[:, 0, 0:1])
        nc.vector.tensor_copy(scratch[0:1, 0:1], bz_sb[0:1, 0:1])
        nc.vector.tensor_copy(scratch[0:1, 1:2], bn_sb[0:1, 1:2])
        nc.vector.tensor_copy(scratch[0:1, 2:3], id_sb[0:1, 0:1])

        seq_pool = ctx.enter_context(tc.tile_pool(name="seqp", bufs=3))
        psum1 = ctx.enter_context(tc.tile_pool(name="psum1", bufs=2, space="PSUM"))
        # per-chunk input-projection deltas, time-major [128, step, m*BC+b].
        # fp32 master plus bf16 hi/lo split (the injection matmuls are bf16:
        # fp32 matmuls measure ~450ns each, bf16 ~27ns; hi+lo keeps the
        # telescoped sum exact to ~2^-18).
        dz_pool = ctx.enter_context(tc.tile_pool(name="dzp", bufs=3))
        dn_pool = ctx.enter_context(tc.tile_pool(name="dnp", bufs=3))

        def chunk_dma(c):
            # no pe_touch here: a junk matmul gated on this DMA would
            # head-of-line-block the PE queue for the DMA's full latency.
            csl = slice(c * PCOL, (c + 1) * PCOL)
            sq = seq_pool.tile([128, CH, PCOL], BF16, tag="sq")
            nc.sync.dma_start(sq[:], seqT[:, csl].rearrange("(c p) r -> p c r", p=128))
            return sq

        def new_delta_tiles():
            dz = dz_pool.tile([128, CSTEPS, D], F32, tag="dz")
            dn = dn_pool.tile([128, CSTEPS, D], F32, tag="dn")
            dzh = dz_pool.tile([128, CSTEPS, D], BF16, tag="dzh")
            dzl = dz_pool.tile([128, CSTEPS, D], BF16, tag="dzl")
            dnh = dn_pool.tile([128, CSTEPS, D], BF16, tag="dnh")
            dnl = dn_pool.tile([128, CSTEPS, D], BF16, tag="dnl")
            return (dz, dn, dzh, dzl, dnh, dnl)

        def p1_group_mm(sq, ps, g, k):
            m = g % CH
            w_sb = wiz_sb if g < CH else win_sb
            nc.tensor.matmul(
                ps[:],
                w_sb[:, k, m * 128 : (m + 1) * 128],
                sq[:, k, :],
                start=(k == 0),
                stop=(k == CH - 1),
            )

        izsb_pool = ctx.enter_context(tc.tile_pool(name="izsb", bufs=2))

        def p1_group_delta_thunks(ps, c, g, dzdn):
            """Per-chunk delta pipeline for group g, as 6 single-op thunks.

            ps columns are (step, batch) flat: col = s*BC + b, s in [0, CSTEPS).
            d[s] = proj[s] - proj[s-1]; cross-chunk boundary via izlast[g];
            chunk 0 uses the (combined) bias instead: d[0] = proj[0] + b.
            DVE reads at most one PSUM input per op, so ps is staged to SBUF.
            The fp32 delta is then split into bf16 hi + lo for injection.
            """
            m = g % CH
            dst = dzdn[0] if g < CH else dzdn[1]
            hi = dzdn[2] if g < CH else dzdn[4]
            lo = dzdn[3] if g < CH else dzdn[5]
            msl = slice(m * BC, (m + 1) * BC)
            izsb = izsb_pool.tile([128, PCOL], F32, tag="izsb")

            # the two big copies run on the Scalar engine's idle tail; DVE
            # keeps only the two-tensor-input subs.
            def t_copy():
                nc.scalar.copy(izsb[:], ps[:])

            def t_shift_sub():
                nc.vector.tensor_sub(
                    dst[:, 1:CSTEPS, msl],
                    izsb[:, BC:PCOL],
                    izsb[:, 0 : PCOL - BC],
                )

            def t_boundary():
                if c == 0:
                    b_sb = bz_sb if g < CH else bn_sb
                    nc.vector.tensor_scalar_add(
                        dst[:, 0, msl], izsb[:, 0:BC], b_sb[:, m : m + 1]
                    )
                else:
                    nc.vector.tensor_sub(
                        dst[:, 0, msl], izsb[:, 0:BC], izlast[:, g, :]
                    )

            def t_izlast():
                nc.vector.tensor_copy(izlast[:, g, :], izsb[:, PCOL - BC : PCOL])

            def t_hi():
                nc.scalar.copy(hi[:, :, msl], dst[:, :, msl])

            def t_lo():
                nc.vector.tensor_sub(lo[:, :, msl], dst[:, :, msl], hi[:, :, msl])

            return [t_copy, t_shift_sub, t_boundary, t_izlast, t_hi, t_lo]

        def p1_group_delta(ps, c, g, dzdn):
            for th in p1_group_delta_thunks(ps, c, g, dzdn):
                th()

        delta_tiles = [new_delta_tiles()]
        for c in range(PRO):
            if c > 0:
                delta_tiles.append(new_delta_tiles())
            sq = chunk_dma(c)
            for g in range(2 * CH):
                ps = psum1.tile([128, PCOL], F32, tag="ps1")
                for k in range(CH):
                    p1_group_mm(sq, ps, g, k)
                p1_group_delta(ps, c, g, delta_tiles[c])
        # prefetch one block ahead: chunk pipe_c's DMA is issued a full block
        # before its phase-1 matmuls run, so they never wait on it.
        sq_next = chunk_dma(PRO) if PRO < NT1 else None

        with (
            tc.tile_pool(name="ht2", bufs=2) as ht_pool,
            tc.tile_pool(name="st2", bufs=2) as state_pool,
            tc.tile_pool(name="ew2", bufs=2) as ew_pool,
            tc.tile_pool(name="accp", bufs=1, space="PSUM") as accp,
        ):
            # zero bf16 tile: moving operand for the step-0 matmuls
            z0 = state_pool.tile([128, D], BF16, tag="z0")
            nc.vector.memset(z0[:], 0.0)
            h0 = state_pool.tile([128, D], F32, tag="h0")
            nc.vector.memset(h0[:], 0.0)
            h = h0
            d2_prev = z0

            # persistent cross-step accumulators: ps_t = inproj_t + W @ h_{t-1}
            psn_acc = accp.tile([128, D], F32, tag="psn_acc")
            psz_acc = accp.tile([128, D], F32, tag="psz_acc")

            p1_sq = None
            p1_ps = None
            p1_pending = []  # deferred (ps, c, g) delta emissions

            for blk in range(NBLK):
                htb = ht_pool.tile([128, BLK, D], F32, tag="htb")
                _, _, dzh_blk, dzl_blk, dnh_blk, dnl_blk = delta_tiles[blk]
                pipe_c = blk + PRO
                if pipe_c < NT1:
                    p1_sq = sq_next
                    if pipe_c + 1 < NT1:
                        sq_next = chunk_dma(pipe_c + 1)
                    delta_tiles.append(new_delta_tiles())

                for tl in range(BLK):
                    t = blk * BLK + tl
                    # inject input-projection deltas (bf16 hi/lo identity
                    # matmuls); n first: gated only on the earlier tanh read.
                    for acc, hi_t, lo_t in (
                        (psn_acc, dnh_blk, dnl_blk),
                        (psz_acc, dzh_blk, dzl_blk),
                    ):
                        nc.tensor.matmul(
                            acc[:],
                            id_sb[:],
                            hi_t[:, tl, :],
                            start=(t == 0),
                            stop=False,
                            skip_group_check=True,
                        )
                        nc.tensor.matmul(
                            acc[:],
                            id_sb[:],
                            lo_t[:, tl, :],
                            start=False,
                            stop=False,
                            skip_group_check=True,
                        )
                    # recurrent matmuls: n gate first (tanh unblocks at half
                    # burst), z gate second.
                    for w_sb, ps in ((whn_sb, psn_acc), (whz_sb, psz_acc)):
                        for m in range(CH):
                            for k in range(CH):
                                nc.tensor.matmul(
                                    ps[:, m * BC : (m + 1) * BC],
                                    w_sb[:, k, m * 128 : (m + 1) * 128],
                                    d2_prev[:, k * BC : (k + 1) * BC],
                                    start=False,
                                    stop=(t == T - 1),
                                    skip_group_check=True,
                                )
                    # phase-1 interleave: one matmul per step fills the tail
                    if pipe_c < NT1 and tl < 8 * CH:
                        g, k = tl // CH, tl % CH
                        if k == 0:
                            p1_ps = psum1.tile([128, PCOL], F32, tag="ps1")
                        p1_group_mm(p1_sq, p1_ps, g, k)
                        if k == CH - 1:
                            # eligible from the NEXT step on: the group's last
                            # phase-1 matmul only runs in this step's tail, and
                            # t_copy waiting on it would head-of-line-block the
                            # DVE queue.
                            p1_pending.extend(
                                (t + 1, th)
                                for th in p1_group_delta_thunks(
                                    p1_ps, pipe_c, g, delta_tiles[pipe_c]
                                )
                            )

                    # n path: tanh reads the accumulator straight from PSUM
                    nt = ew_pool.tile([128, D], BF16, tag="nt")
                    nc.scalar.activation(nt[:], psn_acc[:], AF.Tanh)
                    d = ew_pool.tile([128, D], BF16, tag="d")
                    nc.vector.tensor_sub(d[:], nt[:], h[:])
                    # z path: h_new = h + sigmoid(-tz)*(n - h)
                    zc = ew_pool.tile([128, D], BF16, tag="zc")
                    nc.scalar.activation(zc[:], psz_acc[:], AF.Sigmoid, scale=-1.0)
                    d2 = ew_pool.tile([128, D], BF16, tag="d2")
                    nc.vector.tensor_mul(d2[:], zc[:], d[:])
                    hn = htb[:, tl, :]
                    nc.gpsimd.tensor_add(hn, h[:], d2[:])
                    emitted = 0
                    while p1_pending and p1_pending[0][0] <= t and emitted < 2:
                        p1_pending.pop(0)[1]()
                        emitted += 1
                    h = hn
                    d2_prev = d2

                nc.sync.dma_start(HT[:, blk * BLK : (blk + 1) * BLK, :], htb[:])

    nc.compile()
    return nc


_CACHE: dict = {}


def _get_nc(T, BC):
    key = (T, BC)
    if key not in _CACHE:
        _CACHE[key] = build_nc(T, BC)
    return _CACHE[key]


def _in_maps(inputs, T, BC):
    f32 = np.float32
    wizT = np.ascontiguousarray(np.asarray(inputs["W_iz"], f32).T.astype(NPBF16))
    winT = np.ascontiguousarray(np.asarray(inputs["W_in"], f32).T.astype(NPBF16))
    whzT = np.ascontiguousarray(np.asarray(inputs["W_hz"], f32).T.astype(NPBF16))
    whnT = np.ascontiguousarray(np.asarray(inputs["W_hn"], f32).T.astype(NPBF16))
    biasz = np.ascontiguousarray(
        (np.asarray(inputs["b_iz"], f32) + np.asarray(inputs["b_hz"], f32)).reshape(CH, 128).T
    )
    biasn = np.ascontiguousarray(
        (np.asarray(inputs["b_in"], f32) + np.asarray(inputs["b_hn"], f32)).reshape(CH, 128).T
    )
    ident = np.eye(128, dtype=f32).astype(NPBF16)
    seq = np.asarray(inputs["seq"], f32)
    in_maps = []
    for c in range(NCORES):
        shard = seq[:, c * BC : (c + 1) * BC, :].reshape(T * BC, DIN)
        seqT = np.ascontiguousarray(shard.T.astype(NPBF16))
        in_maps.append(
            {
                "seqT": seqT,
                "wizT": wizT,
                "winT": winT,
                "whzT": whzT,
                "whnT": whnT,
                "biasz": biasz,
                "biasn": biasn,
                "ident": ident,
            }
        )
    return in_maps


def _assemble(results, T, BC):
    out = np.empty((T, NCORES * BC, DH), np.float32)
    for c in range(NCORES):
        HT = np.asarray(results[c]["HT"], dtype=np.float32)  # [128, T, CH*BC]
        # HT[p, t, m*BC+b] -> out[t, b, m*128+p]
        Hc = HT.reshape(128, T, CH, BC).transpose(1, 3, 2, 0).reshape(T, BC, DH)
        out[:, c * BC : (c + 1) * BC, :] = Hc
    return out[None]


def kernel(seq, W_iz, b_iz, W_in, b_in, W_hz, b_hz, W_hn, b_hn):
    seq = np.asarray(seq)
    T, B, _ = seq.shape
    BC = B // NCORES
    nc = _get_nc(T, BC)
    in_maps = _in_maps(
        dict(seq=seq, W_iz=W_iz, b_iz=b_iz, W_in=W_in, b_in=b_in,
             W_hz=W_hz, b_hz=b_hz, W_hn=W_hn, b_hn=b_hn),
        T, BC,
    )
    res = bass_utils.run_bass_kernel_spmd(nc, in_maps, list(range(NCORES)))
    return _assemble(res.results, T, BC)


def traced_run(inputs):
    seq = np.asarray(inputs["seq"])
    T, B, _ = seq.shape
    BC = B // NCORES
    nc = _get_nc(T, BC)
    in_maps = _in_maps(inputs, T, BC)
    return bass_utils.run_bass_kernel_spmd(
        nc, in_maps, list(range(NCORES)), trace=True
    )
